# revision 4
# baseline (speedup 1.0000x reference)
"""Multi-head attention (B=2, S=2048, D=1024, H=16) on 8 Trainium2 cores.

Sharding: core i -> batch i//4, head-group i%4 (4 heads = 2 pairs of 2).
Per core: project q/k/v for its 4 heads (fp32r matmuls), attention with
2-head row-packed score matmuls, softmax via exp + ones-column row-sum
(no max subtraction; scores ~N(0,1) after 1/8 scale), output projection
against the core's 256-row slice of Wo. Host sums 16 partials and adds
(bv @ Wo + bo) once (softmax rows sum to 1, so bv folds into a constant).

All matmul inputs are declared float32r end-to-end (the BIR verifier
requires producers of fp32r-matmul operands to emit fp32r).
"""

import sys

import numpy as np

try:
    import concourse.bacc as bacc
except ImportError:  # grading dir may not have the repo on sys.path
    sys.path.insert(0, "/opt/trn_rl_repo")
    import concourse.bacc as bacc

import concourse.mybir as mybir
import concourse.tile as tile
from concourse import bass_utils

B, S, D, H, DH = 2, 2048, 1024, 16, 64
F32 = mybir.dt.float32
R32 = mybir.dt.float32r
EXP = mybir.ActivationFunctionType.Exp


def _emit(nc, aps):
    xq, xk, xv = aps["xqT"], aps["xkT"], aps["xvT"]
    out_aps = [aps["out_a"], aps["out_b"]]

    with tile.TileContext(nc) as tc, \
         nc.allow_low_precision(reason="fp32r matmul input pipeline"):
        with tc.tile_pool(name="persist", bufs=1, space="SBUF") as sb, \
             tc.tile_pool(name="xstream", bufs=3, space="SBUF") as xp, \
             tc.tile_pool(name="pexp", bufs=4, space="SBUF") as pa_pool, \
             tc.tile_pool(name="zpool", bufs=2, space="SBUF") as z_pool, \
             tc.tile_pool(name="bpool", bufs=2, space="SBUF") as bsc_pool, \
             tc.tile_pool(name="obpool", bufs=3, space="SBUF") as ob_pool:

            wq_sb = sb.tile([128, 2048], R32)
            wk_sb = sb.tile([128, 2048], R32)
            wv_sb = sb.tile([128, 2048], R32)
            wo_sb = sb.tile([128, 2048], R32)
            bqT_sb = sb.tile([128, 2], F32)
            bkT_sb = sb.tile([128, 2], F32)
            ones_sb = sb.tile([128, 64], R32)
            qT_sb = sb.tile([128, 4096], R32)
            kT_sb = sb.tile([128, 4096], R32)
            attnT_sb = sb.tile([128, 4096], R32)
            vaug_sb = sb.tile([128, 16 * 260], R32)

            for dc in range(8):
                nc.sync.dma_start(wq_sb[:, dc * 256:(dc + 1) * 256],
                                  aps["wq"][dc * 128:(dc + 1) * 128, :])
                nc.sync.dma_start(wk_sb[:, dc * 256:(dc + 1) * 256],
                                  aps["wk"][dc * 128:(dc + 1) * 128, :])
                nc.sync.dma_start(wv_sb[:, dc * 256:(dc + 1) * 256],
                                  aps["wv"][dc * 128:(dc + 1) * 128, :])
            for p in range(2):
                nc.sync.dma_start(wo_sb[:, p * 1024:(p + 1) * 1024],
                                  aps["wo"][p * 128:(p + 1) * 128, :])
            nc.sync.dma_start(bqT_sb[:], aps["bqT"][:])
            nc.sync.dma_start(bkT_sb[:], aps["bkT"][:])
            # memset cannot target fp32r; stage 1.0s through an F32 tile.
            onesF = sb.tile([128, 64], F32)
            nc.vector.memset(onesF[:], 1.0)
            nc.vector.tensor_copy(ones_sb[:], onesF[:])
            vones = vaug_sb[:].rearrange("p (b g) -> p b g", g=65)[:, :, 64:65]
            nc.vector.tensor_copy(vones, onesF[:].unsqueeze(2))

            # ---- projections (PSUM: ring of 8 banks, one tag) ----
            with tc.tile_pool(name="projp", bufs=8, space="PSUM") as pp:
                # v: v_aug[j][s(128), h*65:h*65+64] = (x @ Wv) chunk; col h*65+64 stays 1.0
                for rnd in range(2):
                    vps = [pp.tile([128, 256], F32, tag="pp", name=f"vp{rnd}{s8}")
                           for s8 in range(8)]
                    for dc in range(8):
                        xt = xp.tile([128, 2048], R32, tag="x", name=f"xv{rnd}{dc}")
                        nc.sync.dma_start(xt[:], xv[dc * 128:(dc + 1) * 128, :])
                        for s8 in range(8):
                            sc16 = rnd * 8 + s8
                            nc.tensor.matmul(
                                vps[s8][:],
                                xt[:, sc16 * 128:(sc16 + 1) * 128],
                                wv_sb[:, dc * 256:(dc + 1) * 256],
                                start=(dc == 0), stop=(dc == 7))
                    for s8 in range(8):
                        sc16 = rnd * 8 + s8
                        for h in range(4):
                            nc.vector.tensor_copy(
                                vaug_sb[:, sc16 * 260 + h * 65:sc16 * 260 + h * 65 + 64],
                                vps[s8][:, h * 64:(h + 1) * 64])

                # q/k: qT[dh(128 per pair), s] = (Wq_pair)^T x^T + bias
                for which, xin, w_sb, bT_sb, dst in (
                        ("q", xq, wq_sb, bqT_sb, qT_sb),
                        ("k", xk, wk_sb, bkT_sb, kT_sb)):
                    qps = [pp.tile([128, 512], F32, tag="pp", name=f"{which}p{j}")
                           for j in range(8)]
                    for dc in range(8):
                        xt = xp.tile([128, 2048], R32, tag="x", name=f"x{which}{dc}")
                        nc.sync.dma_start(xt[:], xin[dc * 128:(dc + 1) * 128, :])
                        for cc in range(2):
                            for sc in range(4):
                                nc.tensor.matmul(
                                    qps[cc * 4 + sc][:],
                                    w_sb[:, dc * 256 + cc * 128:dc * 256 + cc * 128 + 128],
                                    xt[:, sc * 512:(sc + 1) * 512],
                                    start=(dc == 0), stop=(dc == 7))
                    for cc in range(2):
                        for sc in range(4):
                            nc.vector.tensor_scalar_add(
                                dst[:, cc * 2048 + sc * 512:cc * 2048 + sc * 512 + 512],
                                qps[cc * 4 + sc][:],
                                bT_sb[:, cc:cc + 1])

            # ---- attention + output projection ----
            with tc.tile_pool(name="sp", bufs=3, space="PSUM") as sp, \
                 tc.tile_pool(name="opp", bufs=3, space="PSUM") as op_pool, \
                 tc.tile_pool(name="outp", bufs=2, space="PSUM") as outp:
                for p in range(2):
                    for Q in range(4):
                        qb = p * 2048 + Q * 512
                        poA = op_pool.tile([128, 512], F32, tag="o", name=f"poA{p}{Q}")
                        poB = op_pool.tile([128, 512], F32, tag="o", name=f"poB{p}{Q}")
                        for j in range(16):
                            kb = p * 2048 + j * 128
                            sA = sp.tile([128, 512], F32, tag="s", name=f"sA{p}{Q}{j}")
                            sB = sp.tile([128, 512], F32, tag="s", name=f"sB{p}{Q}{j}")
                            nc.tensor.matmul(sA[:], kT_sb[0:64, kb:kb + 128],
                                             qT_sb[0:64, qb:qb + 512],
                                             start=True, stop=True)
                            nc.tensor.matmul(sB[:], kT_sb[64:128, kb:kb + 128],
                                             qT_sb[64:128, qb:qb + 512],
                                             start=True, stop=True)
                            pA = pa_pool.tile([128, 512], R32, tag="pa", name=f"pA{p}{Q}{j}")
                            pB = pa_pool.tile([128, 512], R32, tag="pa", name=f"pB{p}{Q}{j}")
                            nc.scalar.activation(pA[:], sA[:], EXP, scale=0.125)
                            nc.scalar.activation(pB[:], sB[:], EXP, scale=0.125)
                            va = j * 260 + (2 * p) * 65
                            vb = j * 260 + (2 * p + 1) * 65
                            nc.tensor.matmul(poA[0:65, :], vaug_sb[:, va:va + 65],
                                             pA[:], start=(j == 0), stop=(j == 15))
                            nc.tensor.matmul(poB[0:65, :], vaug_sb[:, vb:vb + 65],
                                             pB[:], start=(j == 0), stop=(j == 15))
                        z = z_pool.tile([128, 1024], R32, tag="z", name=f"z{p}{Q}")
                        nc.vector.reciprocal(z[64:65, 0:512], poA[64:65, :])
                        nc.vector.reciprocal(z[64:65, 512:1024], poB[64:65, :])
                        zbA = sp.tile([128, 512], F32, tag="s", name=f"zbA{p}{Q}")
                        zbB = sp.tile([128, 512], F32, tag="s", name=f"zbB{p}{Q}")
                        nc.tensor.matmul(zbA[0:64, :], ones_sb[64:65, :],
                                         z[64:65, 0:512], start=True, stop=True)
                        nc.tensor.matmul(zbB[0:64, :], ones_sb[64:65, :],
                                         z[64:65, 512:1024], start=True, stop=True)
                        # DVE tensor_tensor cannot read two PSUM operands;
                        # stage the 1/Z broadcast through SBUF.
                        zbs = z_pool.tile([64, 1024], F32, tag="zbs", name=f"zbs{p}{Q}")
                        nc.vector.tensor_copy(zbs[:, 0:512], zbA[0:64, :])
                        nc.vector.tensor_copy(zbs[:, 512:1024], zbB[0:64, :])
                        nc.vector.tensor_mul(attnT_sb[0:64, qb:qb + 512],
                                             poA[0:64, :], zbs[:, 0:512])
                        bsc = bsc_pool.tile([64, 512], R32, tag="b", name=f"bsc{p}{Q}")
                        nc.vector.tensor_mul(bsc[:], poB[0:64, :], zbs[:, 512:1024])
                        nc.sync.dma_start(attnT_sb[64:128, qb:qb + 512], bsc[:])

                    for t in range(16):
                        for n in range(2):
                            up = outp.tile([128, 512], F32, tag="u", name=f"u{p}{t}{n}")
                            nc.tensor.matmul(
                                up[:],
                                attnT_sb[:, p * 2048 + t * 128:p * 2048 + t * 128 + 128],
                                wo_sb[:, p * 1024 + n * 512:p * 1024 + n * 512 + 512],
                                start=True, stop=True)
                            ob = ob_pool.tile([128, 512], F32, tag="ob", name=f"ob{p}{t}{n}")
                            nc.vector.tensor_copy(ob[:], up[:])
                            nc.sync.dma_start(
                                out_aps[p][t * 128:(t + 1) * 128, n * 512:(n + 1) * 512],
                                ob[:])


_NC = None


def _get_nc():
    global _NC
    if _NC is None:
        nc = bacc.Bacc("TRN2", target_bir_lowering=False, debug=False,
                       enable_asserts=False, num_devices=8)
        aps = {}
        for nm, shp in [("xqT", (D, S)), ("xkT", (D, S)), ("xvT", (D, S)),
                        ("wq", (D, 256)), ("wk", (D, 256)), ("wv", (D, 256)),
                        ("wo", (256, D))]:
            aps[nm] = nc.dram_tensor(nm, shp, R32, kind="ExternalInput").ap()
        for nm, shp in [("bqT", (128, 2)), ("bkT", (128, 2))]:
            aps[nm] = nc.dram_tensor(nm, shp, F32, kind="ExternalInput").ap()
        for nm in ("out_a", "out_b"):
            aps[nm] = nc.dram_tensor(nm, (S, D), F32, kind="ExternalOutput").ap()
        _emit(nc, aps)
        nc.compile()
        _NC = nc
    return _NC


def _run(inputs, trace=False):
    nc = _get_nc()
    f = np.float32
    q = np.asarray(inputs["query"], dtype=f)
    k = np.asarray(inputs["key"], dtype=f)
    v = np.asarray(inputs["value"], dtype=f)
    Wq = np.asarray(inputs["Wq"], dtype=f)
    Wk = np.asarray(inputs["Wk"], dtype=f)
    Wv = np.asarray(inputs["Wv"], dtype=f)
    Wo = np.asarray(inputs["Wo"], dtype=f)
    bq = np.asarray(inputs["bq"], dtype=f)
    bk = np.asarray(inputs["bk"], dtype=f)
    bv = np.asarray(inputs["bv"], dtype=f)
    bo = np.asarray(inputs["bo"], dtype=f)

    xT = {b: (np.ascontiguousarray(q[b].T),
              np.ascontiguousarray(k[b].T),
              np.ascontiguousarray(v[b].T)) for b in range(B)}
    in_maps = []
    for i in range(8):
        b, hg = divmod(i, 4)
        c0 = hg * 256
        in_maps.append({
            "xqT": xT[b][0], "xkT": xT[b][1], "xvT": xT[b][2],
            "wq": np.ascontiguousarray(Wq[:, c0:c0 + 256]),
            "wk": np.ascontiguousarray(Wk[:, c0:c0 + 256]),
            "wv": np.ascontiguousarray(Wv[:, c0:c0 + 256]),
            "bqT": np.ascontiguousarray(bq[c0:c0 + 256].reshape(2, 128).T),
            "bkT": np.ascontiguousarray(bk[c0:c0 + 256].reshape(2, 128).T),
            "wo": np.ascontiguousarray(Wo[c0:c0 + 256, :]),
        })

    res = bass_utils.run_bass_kernel_spmd(nc, in_maps, core_ids=list(range(8)),
                                          trace=trace)
    out = np.zeros((B, S, D), dtype=f)
    for i in range(8):
        out[i // 4] += np.asarray(res.results[i]["out_a"])
        out[i // 4] += np.asarray(res.results[i]["out_b"])
    out += (bv @ Wo + bo)[None, None, :]
    return out, res


def kernel(**inputs):
    out, _ = _run(inputs, trace=False)
    return out


# revision 12
# speedup vs baseline: 1.0623x; 1.0623x over previous
"""Multi-head attention (B=2, S=2048, D=1024, H=16) on 8 Trainium2 cores.

Sharding: core i -> batch i//4, head-group i%4 (4 heads = 2 pairs of 2).
v2 layout: x streamed q->k->v with xv resident in an 8-slot SBUF ring
(v read once); q/k bias-evac on the scalar engine (idle in phase 1);
scores for both heads of a key-chunk share one [128,1024] PSUM tile so
softmax needs a single exp per chunk; output projection is interleaved
per (pair, query-block) to spread output DMA across the attention phase.
Host sums 16 partials and adds (bv @ Wo + bo) once.
"""

import sys

import numpy as np

try:
    import concourse.bacc as bacc
except ImportError:  # grading dir may not have the repo on sys.path
    sys.path.insert(0, "/opt/trn_rl_repo")
    import concourse.bacc as bacc

import concourse.mybir as mybir
import concourse.tile as tile
from concourse import bass_utils

B, S, D, H, DH = 2, 2048, 1024, 16, 64
F32 = mybir.dt.float32
R32 = mybir.dt.float32r
XDT = R32  # dtype of streamed x and wq/wk/wv
EXP = mybir.ActivationFunctionType.Exp


def _emit(nc, aps):
    xq, xk, xv = aps["xqT"], aps["xkT"], aps["xvT"]
    out_aps = [aps["out_a"], aps["out_b"]]

    with tile.TileContext(nc) as tc, \
         nc.allow_low_precision(reason="fp32r matmul input pipeline"):
        with tc.tile_pool(name="persist", bufs=1, space="SBUF") as sb, \
             tc.tile_pool(name="xstream", bufs=8, space="SBUF") as xp, \
             tc.tile_pool(name="pexp", bufs=3, space="SBUF") as pa_pool, \
             tc.tile_pool(name="zpool", bufs=2, space="SBUF") as z_pool, \
             tc.tile_pool(name="bpool", bufs=2, space="SBUF") as bsc_pool, \
             tc.tile_pool(name="obpool", bufs=2, space="SBUF") as ob_pool:

            wq_sb = sb.tile([128, 2048], XDT)
            wk_sb = sb.tile([128, 2048], XDT)
            wv_sb = sb.tile([128, 2048], XDT)
            wo_sb = sb.tile([128, 2048], R32)
            bqT_sb = sb.tile([128, 2], F32)
            bkT_sb = sb.tile([128, 2], F32)
            ones_sb = sb.tile([128, 64], R32)
            qT_sb = sb.tile([128, 4096], R32)
            kT_sb = sb.tile([128, 4096], R32)
            attnT_sb = sb.tile([128, 2048], R32)
            vaug_sb = sb.tile([128, 16 * 260], R32)

            for dc in range(8):
                nc.sync.dma_start(wq_sb[:, dc * 256:(dc + 1) * 256],
                                  aps["wq"][dc * 128:(dc + 1) * 128, :])
            nc.sync.dma_start(bqT_sb[:], aps["bqT"][:])
            # memset cannot target fp32r; stage 1.0s through an F32 tile.
            onesF = sb.tile([128, 64], F32)
            nc.vector.memset(onesF[:], 1.0)
            nc.vector.tensor_copy(ones_sb[:], onesF[:])
            # v_aug row layout per key-chunk j (260 cols): 4 x [v(64) 1]
            vj = vaug_sb[:].rearrange("p (j r) -> p j r", r=260)
            for c in (64, 129, 194, 259):
                nc.vector.tensor_copy(vj[:, :, c:c + 1],
                                      onesF[:, 0:16].unsqueeze(2))

            # ---- q/k projections (PSUM: ring of 8 banks, one tag) ----
            with tc.tile_pool(name="projp", bufs=8, space="PSUM") as pp:
                for which, xin, w_sb, bT_sb, dst in (
                        ("q", xq, wq_sb, bqT_sb, qT_sb),
                        ("k", xk, wk_sb, bkT_sb, kT_sb)):
                    qps = [pp.tile([128, 512], F32, tag="pp", name=f"{which}p{j}")
                           for j in range(8)]
                    for dc in range(8):
                        xt = xp.tile([128, 2048], XDT, tag="x", name=f"x{which}{dc}")
                        nc.sync.dma_start(xt[:], xin[dc * 128:(dc + 1) * 128, :])
                        for cc in range(2):
                            for sc in range(4):
                                nc.tensor.matmul(
                                    qps[cc * 4 + sc][:],
                                    w_sb[:, dc * 256 + cc * 128:dc * 256 + cc * 128 + 128],
                                    xt[:, sc * 512:(sc + 1) * 512],
                                    start=(dc == 0), stop=(dc == 7))
                    for cc in range(2):
                        for sc in range(4):
                            nc.scalar.add(
                                dst[:, cc * 2048 + sc * 512:cc * 2048 + sc * 512 + 512],
                                qps[cc * 4 + sc][:],
                                bT_sb[:, cc:cc + 1])
                    if which == "q":
                        for dc in range(8):
                            nc.sync.dma_start(wk_sb[:, dc * 256:(dc + 1) * 256],
                                              aps["wk"][dc * 128:(dc + 1) * 128, :])
                        nc.sync.dma_start(bkT_sb[:], aps["bkT"][:])

                for dc in range(8):
                    nc.sync.dma_start(wv_sb[:, dc * 256:(dc + 1) * 256],
                                      aps["wv"][dc * 128:(dc + 1) * 128, :])
                for p in range(2):
                    nc.sync.dma_start(wo_sb[:, p * 1024:(p + 1) * 1024],
                                      aps["wo"][p * 128:(p + 1) * 128, :])
                xvt = []
                for dc in range(8):
                    xt = xp.tile([128, 2048], XDT, tag="x", name=f"xv{dc}")
                    nc.sync.dma_start(xt[:], xv[dc * 128:(dc + 1) * 128, :])
                    xvt.append(xt)

            # ---- v projection + attention + output projection ----
            with tc.tile_pool(name="sp", bufs=2, space="PSUM") as sp, \
                 tc.tile_pool(name="opp", bufs=2, space="PSUM") as opp, \
                 tc.tile_pool(name="vup", bufs=2, space="PSUM") as vup:
                # v: 16 rounds, 1 PSUM bank each, reading the resident xv ring;
                # single strided evac copy per round.
                for j in range(16):
                    vp = vup.tile([128, 256], F32, tag="v", name=f"vp{j}")
                    for dc in range(8):
                        nc.tensor.matmul(vp[:],
                                         xvt[dc][:, j * 128:(j + 1) * 128],
                                         wv_sb[:, dc * 256:(dc + 1) * 256],
                                         start=(dc == 0), stop=(dc == 7))
                    base = j * 260
                    dst = vaug_sb[:, base:base + 260] \
                        .rearrange("p (g c) -> p g c", c=65)[:, :, 0:64]
                    nc.vector.tensor_copy(
                        dst, vp[:].rearrange("p (g c) -> p g c", c=64))

                for p in range(2):
                    for Q in range(4):
                        qb = p * 2048 + Q * 512
                        poA = opp.tile([128, 512], F32, tag="o", name=f"poA{p}{Q}")
                        poB = opp.tile([128, 512], F32, tag="o", name=f"poB{p}{Q}")
                        for j in range(16):
                            kb = p * 2048 + j * 128
                            sBig = sp.tile([128, 1024], F32, tag="s",
                                           name=f"s{p}{Q}{j}")
                            nc.tensor.matmul(sBig[:, 0:512],
                                             kT_sb[0:64, kb:kb + 128],
                                             qT_sb[0:64, qb:qb + 512],
                                             start=True, stop=True)
                            nc.tensor.matmul(sBig[:, 512:1024],
                                             kT_sb[64:128, kb:kb + 128],
                                             qT_sb[64:128, qb:qb + 512],
                                             start=True, stop=True)
                            pBig = pa_pool.tile([128, 1024], R32, tag="pa",
                                                name=f"pb{p}{Q}{j}")
                            nc.scalar.activation(pBig[:], sBig[:], EXP, scale=0.125)
                            va = j * 260 + (2 * p) * 65
                            vb = j * 260 + (2 * p + 1) * 65
                            nc.tensor.matmul(poA[0:65, :], vaug_sb[:, va:va + 65],
                                             pBig[:, 0:512],
                                             start=(j == 0), stop=(j == 15))
                            nc.tensor.matmul(poB[0:65, :], vaug_sb[:, vb:vb + 65],
                                             pBig[:, 512:1024],
                                             start=(j == 0), stop=(j == 15))
                        z = z_pool.tile([128, 1024], R32, tag="z", name=f"z{p}{Q}")
                        nc.vector.reciprocal(z[64:65, 0:512], poA[64:65, :])
                        nc.vector.reciprocal(z[64:65, 512:1024], poB[64:65, :])
                        zz = sp.tile([128, 1024], F32, tag="s", name=f"zz{p}{Q}")
                        nc.tensor.matmul(zz[0:64, 0:512], ones_sb[64:65, :],
                                         z[64:65, 0:512], start=True, stop=True)
                        nc.tensor.matmul(zz[0:64, 512:1024], ones_sb[64:65, :],
                                         z[64:65, 512:1024], start=True, stop=True)
                        # DVE tensor_tensor cannot read two PSUM operands;
                        # stage the 1/Z broadcast through SBUF.
                        zbs = z_pool.tile([64, 1024], F32, tag="zbs",
                                          name=f"zbs{p}{Q}")
                        nc.vector.tensor_copy(zbs[:], zz[0:64, :])
                        Qb = Q * 512
                        nc.vector.tensor_mul(attnT_sb[0:64, Qb:Qb + 512],
                                             poA[0:64, :], zbs[:, 0:512])
                        bsc = bsc_pool.tile([64, 512], R32, tag="b",
                                            name=f"bsc{p}{Q}")
                        nc.vector.tensor_mul(bsc[:], poB[0:64, :],
                                             zbs[:, 512:1024])
                        nc.sync.dma_start(attnT_sb[64:128, Qb:Qb + 512], bsc[:])
                        for t in range(Q * 4, Q * 4 + 4):
                            for n in range(2):
                                up = vup.tile([128, 512], F32, tag="v",
                                              name=f"u{p}{t}{n}")
                                nc.tensor.matmul(
                                    up[:],
                                    attnT_sb[:, t * 128:(t + 1) * 128],
                                    wo_sb[:, p * 1024 + n * 512:p * 1024 + n * 512 + 512],
                                    start=True, stop=True)
                                ob = ob_pool.tile([128, 512], F32, tag="ob",
                                                  name=f"ob{p}{t}{n}")
                                nc.vector.tensor_copy(ob[:], up[:])
                                nc.sync.dma_start(
                                    out_aps[p][t * 128:(t + 1) * 128,
                                               n * 512:(n + 1) * 512],
                                    ob[:])


_NC = None


def _get_nc():
    global _NC
    if _NC is None:
        nc = bacc.Bacc("TRN2", target_bir_lowering=False, debug=False,
                       enable_asserts=False, num_devices=8)
        aps = {}
        for nm, shp in [("xqT", (D, S)), ("xkT", (D, S)), ("xvT", (D, S)),
                        ("wq", (D, 256)), ("wk", (D, 256)), ("wv", (D, 256))]:
            aps[nm] = nc.dram_tensor(nm, shp, XDT, kind="ExternalInput").ap()
        aps["wo"] = nc.dram_tensor("wo", (256, D), R32, kind="ExternalInput").ap()
        for nm, shp in [("bqT", (128, 2)), ("bkT", (128, 2))]:
            aps[nm] = nc.dram_tensor(nm, shp, F32, kind="ExternalInput").ap()
        for nm in ("out_a", "out_b"):
            aps[nm] = nc.dram_tensor(nm, (S, D), F32, kind="ExternalOutput").ap()
        _emit(nc, aps)
        nc.compile()
        _NC = nc
    return _NC


def _run(inputs, trace=False):
    nc = _get_nc()
    f = np.float32
    q = np.asarray(inputs["query"], dtype=f)
    k = np.asarray(inputs["key"], dtype=f)
    v = np.asarray(inputs["value"], dtype=f)
    Wq = np.asarray(inputs["Wq"], dtype=f)
    Wk = np.asarray(inputs["Wk"], dtype=f)
    Wv = np.asarray(inputs["Wv"], dtype=f)
    Wo = np.asarray(inputs["Wo"], dtype=f)
    bq = np.asarray(inputs["bq"], dtype=f)
    bk = np.asarray(inputs["bk"], dtype=f)
    bv = np.asarray(inputs["bv"], dtype=f)
    bo = np.asarray(inputs["bo"], dtype=f)

    xT = {b: (np.ascontiguousarray(q[b].T),
              np.ascontiguousarray(k[b].T),
              np.ascontiguousarray(v[b].T)) for b in range(B)}
    in_maps = []
    for i in range(8):
        b, hg = divmod(i, 4)
        c0 = hg * 256
        in_maps.append({
            "xqT": xT[b][0], "xkT": xT[b][1], "xvT": xT[b][2],
            "wq": np.ascontiguousarray(Wq[:, c0:c0 + 256]),
            "wk": np.ascontiguousarray(Wk[:, c0:c0 + 256]),
            "wv": np.ascontiguousarray(Wv[:, c0:c0 + 256]),
            "bqT": np.ascontiguousarray(bq[c0:c0 + 256].reshape(2, 128).T),
            "bkT": np.ascontiguousarray(bk[c0:c0 + 256].reshape(2, 128).T),
            "wo": np.ascontiguousarray(Wo[c0:c0 + 256, :]),
        })

    res = bass_utils.run_bass_kernel_spmd(nc, in_maps, core_ids=list(range(8)),
                                          trace=trace)
    out = np.zeros((B, S, D), dtype=f)
    for i in range(8):
        out[i // 4] += np.asarray(res.results[i]["out_a"])
        out[i // 4] += np.asarray(res.results[i]["out_b"])
    out += (bv @ Wo + bo)[None, None, :]
    return out, res


def kernel(**inputs):
    out, _ = _run(inputs, trace=False)
    return out


# revision 13
# speedup vs baseline: 1.0676x; 1.0050x over previous
"""Multi-head attention (B=2, S=2048, D=1024, H=16) on 8 Trainium2 cores.

Sharding: core i -> batch i//4, head-group i%4 (4 heads = 2 pairs of 2).
v2 layout: x streamed q->k->v with xv resident in an 8-slot SBUF ring
(v read once); q/k bias-evac on the scalar engine (idle in phase 1);
scores for both heads of a key-chunk share one [128,1024] PSUM tile so
softmax needs a single exp per chunk; output projection is interleaved
per (pair, query-block) to spread output DMA across the attention phase.
Host sums 16 partials and adds (bv @ Wo + bo) once.
"""

import sys

import numpy as np

try:
    import concourse.bacc as bacc
except ImportError:  # grading dir may not have the repo on sys.path
    sys.path.insert(0, "/opt/trn_rl_repo")
    import concourse.bacc as bacc

import concourse.mybir as mybir
import concourse.tile as tile
from concourse import bass_utils

B, S, D, H, DH = 2, 2048, 1024, 16, 64
F32 = mybir.dt.float32
R32 = mybir.dt.float32r
XDT = R32  # dtype of streamed x and wq/wk/wv
EXP = mybir.ActivationFunctionType.Exp


def _emit(nc, aps):
    xq, xk, xv = aps["xqT"], aps["xkT"], aps["xvT"]
    out_aps = [aps["out_a"], aps["out_b"]]

    with tile.TileContext(nc) as tc, \
         nc.allow_low_precision(reason="fp32r matmul input pipeline"):
        with tc.tile_pool(name="persist", bufs=1, space="SBUF") as sb, \
             tc.tile_pool(name="xstream", bufs=8, space="SBUF") as xp, \
             tc.tile_pool(name="pexp", bufs=3, space="SBUF") as pa_pool, \
             tc.tile_pool(name="zpool", bufs=2, space="SBUF") as z_pool, \
             tc.tile_pool(name="bpool", bufs=2, space="SBUF") as bsc_pool, \
             tc.tile_pool(name="obpool", bufs=2, space="SBUF") as ob_pool:

            wq_sb = sb.tile([128, 2048], XDT)
            wk_sb = sb.tile([128, 2048], XDT)
            wv_sb = sb.tile([128, 2048], XDT)
            wo_sb = sb.tile([128, 2048], R32)
            bqT_sb = sb.tile([128, 2], F32)
            bkT_sb = sb.tile([128, 2], F32)
            ones_sb = sb.tile([128, 64], R32)
            qT_sb = sb.tile([128, 4096], R32)
            kT_sb = sb.tile([128, 4096], R32)
            attnT_sb = sb.tile([128, 2048], R32)
            vaug_sb = sb.tile([128, 16 * 260], R32)

            for dc in range(8):
                nc.sync.dma_start(wq_sb[:, dc * 256:(dc + 1) * 256],
                                  aps["wq"][dc * 128:(dc + 1) * 128, :])
            nc.sync.dma_start(bqT_sb[:], aps["bqT"][:])
            # memset cannot target fp32r; stage 1.0s through an F32 tile.
            onesF = sb.tile([128, 64], F32)
            nc.vector.memset(onesF[:], 1.0)
            nc.vector.tensor_copy(ones_sb[:], onesF[:])
            # v_aug row layout per key-chunk j (260 cols): 4 x [v(64) 1]
            vj = vaug_sb[:].rearrange("p (j r) -> p j r", r=260)
            for c in (64, 129, 194, 259):
                nc.vector.tensor_copy(vj[:, :, c:c + 1],
                                      onesF[:, 0:16].unsqueeze(2))

            # ---- q/k projections (PSUM: ring of 8 banks, one tag) ----
            with tc.tile_pool(name="projp", bufs=8, space="PSUM") as pp:
                for which, xin, w_sb, bT_sb, dst in (
                        ("q", xq, wq_sb, bqT_sb, qT_sb),
                        ("k", xk, wk_sb, bkT_sb, kT_sb)):
                    qps = [pp.tile([128, 512], F32, tag="pp", name=f"{which}p{j}")
                           for j in range(8)]
                    for dc in range(8):
                        xt = xp.tile([128, 2048], XDT, tag="x", name=f"x{which}{dc}")
                        nc.sync.dma_start(xt[:], xin[dc * 128:(dc + 1) * 128, :])
                        for cc in range(2):
                            for sc in range(4):
                                nc.tensor.matmul(
                                    qps[cc * 4 + sc][:],
                                    w_sb[:, dc * 256 + cc * 128:dc * 256 + cc * 128 + 128],
                                    xt[:, sc * 512:(sc + 1) * 512],
                                    start=(dc == 0), stop=(dc == 7))
                    for cc in range(2):
                        for sc in range(4):
                            nc.scalar.add(
                                dst[:, cc * 2048 + sc * 512:cc * 2048 + sc * 512 + 512],
                                qps[cc * 4 + sc][:],
                                bT_sb[:, cc:cc + 1])
                    if which == "q":
                        for dc in range(8):
                            nc.sync.dma_start(wk_sb[:, dc * 256:(dc + 1) * 256],
                                              aps["wk"][dc * 128:(dc + 1) * 128, :])
                        nc.sync.dma_start(bkT_sb[:], aps["bkT"][:])

                for dc in range(8):
                    nc.sync.dma_start(wv_sb[:, dc * 256:(dc + 1) * 256],
                                      aps["wv"][dc * 128:(dc + 1) * 128, :])
                for p in range(2):
                    nc.sync.dma_start(wo_sb[:, p * 1024:(p + 1) * 1024],
                                      aps["wo"][p * 128:(p + 1) * 128, :])
                xvt = []
                for dc in range(8):
                    xt = xp.tile([128, 2048], XDT, tag="x", name=f"xv{dc}")
                    nc.sync.dma_start(xt[:], xv[dc * 128:(dc + 1) * 128, :])
                    xvt.append(xt)

            # ---- v projection + attention + output projection ----
            with tc.tile_pool(name="sp", bufs=2, space="PSUM") as sp, \
                 tc.tile_pool(name="opp", bufs=2, space="PSUM") as opp, \
                 tc.tile_pool(name="vup", bufs=2, space="PSUM") as vup:
                # v: 16 rounds, 1 PSUM bank each, reading the resident xv ring;
                # single strided evac copy per round.
                for j in range(16):
                    vp = vup.tile([128, 256], F32, tag="v", name=f"vp{j}")
                    for dc in range(8):
                        nc.tensor.matmul(vp[:],
                                         xvt[dc][:, j * 128:(j + 1) * 128],
                                         wv_sb[:, dc * 256:(dc + 1) * 256],
                                         start=(dc == 0), stop=(dc == 7))
                    base = j * 260
                    dst = vaug_sb[:, base:base + 260] \
                        .rearrange("p (g c) -> p g c", c=65)[:, :, 0:64]
                    nc.vector.tensor_copy(
                        dst, vp[:].rearrange("p (g c) -> p g c", c=64))

                def outproj(p, Q):
                    for t in range(Q * 4, Q * 4 + 4):
                        for n in range(2):
                            up = vup.tile([128, 512], F32, tag="v",
                                          name=f"u{p}{t}{n}")
                            nc.tensor.matmul(
                                up[:],
                                attnT_sb[:, t * 128:(t + 1) * 128],
                                wo_sb[:, p * 1024 + n * 512:p * 1024 + n * 512 + 512],
                                start=True, stop=True)
                            ob = ob_pool.tile([128, 512], F32, tag="ob",
                                              name=f"ob{p}{t}{n}")
                            nc.vector.tensor_copy(ob[:], up[:])
                            nc.sync.dma_start(
                                out_aps[p][t * 128:(t + 1) * 128,
                                           n * 512:(n + 1) * 512],
                                ob[:])

                # Software pipeline: attnV lags scores/exp by one chunk so the
                # in-order PE queue never waits on the exp of the chunk it just
                # scored; the previous block's output projection is deferred
                # into this block's j-loop (j==4) so block boundaries don't
                # stall the PE either.
                pending = None
                for p in range(2):
                    for Q in range(4):
                        qb = p * 2048 + Q * 512
                        poA = opp.tile([128, 512], F32, tag="o", name=f"poA{p}{Q}")
                        poB = opp.tile([128, 512], F32, tag="o", name=f"poB{p}{Q}")
                        pBigs = [None] * 16

                        def attnv(j, poA=poA, poB=poB, p=p, pBigs=pBigs):
                            va = j * 260 + (2 * p) * 65
                            vb = j * 260 + (2 * p + 1) * 65
                            nc.tensor.matmul(poA[0:65, :], vaug_sb[:, va:va + 65],
                                             pBigs[j][:, 0:512],
                                             start=(j == 0), stop=(j == 15))
                            nc.tensor.matmul(poB[0:65, :], vaug_sb[:, vb:vb + 65],
                                             pBigs[j][:, 512:1024],
                                             start=(j == 0), stop=(j == 15))

                        for j in range(16):
                            kb = p * 2048 + j * 128
                            sBig = sp.tile([128, 1024], F32, tag="s",
                                           name=f"s{p}{Q}{j}")
                            nc.tensor.matmul(sBig[:, 0:512],
                                             kT_sb[0:64, kb:kb + 128],
                                             qT_sb[0:64, qb:qb + 512],
                                             start=True, stop=True)
                            nc.tensor.matmul(sBig[:, 512:1024],
                                             kT_sb[64:128, kb:kb + 128],
                                             qT_sb[64:128, qb:qb + 512],
                                             start=True, stop=True)
                            pBigs[j] = pa_pool.tile([128, 1024], R32, tag="pa",
                                                    name=f"pb{p}{Q}{j}")
                            nc.scalar.activation(pBigs[j][:], sBig[:], EXP,
                                                 scale=0.125)
                            if j > 0:
                                attnv(j - 1)
                            if j == 4 and pending is not None:
                                pending()
                                pending = None
                        attnv(15)
                        z = z_pool.tile([128, 1024], R32, tag="z", name=f"z{p}{Q}")
                        nc.vector.reciprocal(z[64:65, 0:512], poA[64:65, :])
                        nc.vector.reciprocal(z[64:65, 512:1024], poB[64:65, :])
                        zz = sp.tile([128, 1024], F32, tag="s", name=f"zz{p}{Q}")
                        nc.tensor.matmul(zz[0:64, 0:512], ones_sb[64:65, :],
                                         z[64:65, 0:512], start=True, stop=True)
                        nc.tensor.matmul(zz[0:64, 512:1024], ones_sb[64:65, :],
                                         z[64:65, 512:1024], start=True, stop=True)
                        # DVE tensor_tensor cannot read two PSUM operands;
                        # stage the 1/Z broadcast through SBUF.
                        zbs = z_pool.tile([64, 1024], F32, tag="zbs",
                                          name=f"zbs{p}{Q}")
                        nc.vector.tensor_copy(zbs[:], zz[0:64, :])
                        Qb = Q * 512
                        nc.vector.tensor_mul(attnT_sb[0:64, Qb:Qb + 512],
                                             poA[0:64, :], zbs[:, 0:512])
                        bsc = bsc_pool.tile([64, 512], R32, tag="b",
                                            name=f"bsc{p}{Q}")
                        nc.vector.tensor_mul(bsc[:], poB[0:64, :],
                                             zbs[:, 512:1024])
                        nc.sync.dma_start(attnT_sb[64:128, Qb:Qb + 512], bsc[:])
                        pending = (lambda p=p, Q=Q: outproj(p, Q))
                pending()


_NC = None


def _get_nc():
    global _NC
    if _NC is None:
        nc = bacc.Bacc("TRN2", target_bir_lowering=False, debug=False,
                       enable_asserts=False, num_devices=8)
        aps = {}
        for nm, shp in [("xqT", (D, S)), ("xkT", (D, S)), ("xvT", (D, S)),
                        ("wq", (D, 256)), ("wk", (D, 256)), ("wv", (D, 256))]:
            aps[nm] = nc.dram_tensor(nm, shp, XDT, kind="ExternalInput").ap()
        aps["wo"] = nc.dram_tensor("wo", (256, D), R32, kind="ExternalInput").ap()
        for nm, shp in [("bqT", (128, 2)), ("bkT", (128, 2))]:
            aps[nm] = nc.dram_tensor(nm, shp, F32, kind="ExternalInput").ap()
        for nm in ("out_a", "out_b"):
            aps[nm] = nc.dram_tensor(nm, (S, D), F32, kind="ExternalOutput").ap()
        _emit(nc, aps)
        nc.compile()
        _NC = nc
    return _NC


def _run(inputs, trace=False):
    nc = _get_nc()
    f = np.float32
    q = np.asarray(inputs["query"], dtype=f)
    k = np.asarray(inputs["key"], dtype=f)
    v = np.asarray(inputs["value"], dtype=f)
    Wq = np.asarray(inputs["Wq"], dtype=f)
    Wk = np.asarray(inputs["Wk"], dtype=f)
    Wv = np.asarray(inputs["Wv"], dtype=f)
    Wo = np.asarray(inputs["Wo"], dtype=f)
    bq = np.asarray(inputs["bq"], dtype=f)
    bk = np.asarray(inputs["bk"], dtype=f)
    bv = np.asarray(inputs["bv"], dtype=f)
    bo = np.asarray(inputs["bo"], dtype=f)

    xT = {b: (np.ascontiguousarray(q[b].T),
              np.ascontiguousarray(k[b].T),
              np.ascontiguousarray(v[b].T)) for b in range(B)}
    in_maps = []
    for i in range(8):
        b, hg = divmod(i, 4)
        c0 = hg * 256
        in_maps.append({
            "xqT": xT[b][0], "xkT": xT[b][1], "xvT": xT[b][2],
            "wq": np.ascontiguousarray(Wq[:, c0:c0 + 256]),
            "wk": np.ascontiguousarray(Wk[:, c0:c0 + 256]),
            "wv": np.ascontiguousarray(Wv[:, c0:c0 + 256]),
            "bqT": np.ascontiguousarray(bq[c0:c0 + 256].reshape(2, 128).T),
            "bkT": np.ascontiguousarray(bk[c0:c0 + 256].reshape(2, 128).T),
            "wo": np.ascontiguousarray(Wo[c0:c0 + 256, :]),
        })

    res = bass_utils.run_bass_kernel_spmd(nc, in_maps, core_ids=list(range(8)),
                                          trace=trace)
    out = np.zeros((B, S, D), dtype=f)
    for i in range(8):
        out[i // 4] += np.asarray(res.results[i]["out_a"])
        out[i // 4] += np.asarray(res.results[i]["out_b"])
    out += (bv @ Wo + bo)[None, None, :]
    return out, res


def kernel(**inputs):
    out, _ = _run(inputs, trace=False)
    return out


# revision 14
# speedup vs baseline: 1.2349x; 1.1566x over previous
"""Multi-head attention (B=2, S=2048, D=1024, H=16) on 8 Trainium2 cores.

Sharding: core i -> batch i//4, head-group i%4 (4 heads = 2 pairs of 2).
v3: x and wq/wk/wv stream as bf16 in order xq->xv->xk; xv is resident in
an 8-slot ring so the v projection (dc-outer, two half-passes of 8 seq
chunks in the same 8-bank PSUM ring as q/k) consumes each chunk as it
arrives and attention starts right after the k evac.  Scores for both
heads of a key-chunk share one [128,1024] PSUM tile -> a single exp per
chunk on ACT (the attention-phase bottleneck).  attnV lags exp by one
chunk; a block's normalization is deferred to the next block's j==1 and
its output projection to j==4 so block boundaries never stall the PE.
The last query block is split into two 256-wide halves to shrink the
serial tail.  Host sums 16 partials and adds (bv @ Wo + bo) once.
"""

import sys

import numpy as np

try:
    import concourse.bacc as bacc
except ImportError:  # grading dir may not have the repo on sys.path
    sys.path.insert(0, "/opt/trn_rl_repo")
    import concourse.bacc as bacc

import ml_dtypes
import concourse.mybir as mybir
import concourse.tile as tile
from concourse import bass_utils

B, S, D, H, DH = 2, 2048, 1024, 16, 64
F32 = mybir.dt.float32
R32 = mybir.dt.float32r
XDT = mybir.dt.bfloat16  # dtype of streamed x and wq/wk/wv
EXP = mybir.ActivationFunctionType.Exp


def _emit(nc, aps):
    xq, xk, xv = aps["xqT"], aps["xkT"], aps["xvT"]
    out_aps = [aps["out_a"], aps["out_b"]]

    with tile.TileContext(nc) as tc, \
         nc.allow_low_precision(reason="bf16/fp32r matmul input pipeline"):
        with tc.tile_pool(name="persist", bufs=1, space="SBUF") as sb, \
             tc.tile_pool(name="xres", bufs=8, space="SBUF") as xvp, \
             tc.tile_pool(name="xstream", bufs=3, space="SBUF") as xp, \
             tc.tile_pool(name="pexp", bufs=4, space="SBUF") as pa_pool, \
             tc.tile_pool(name="zpool", bufs=2, space="SBUF") as z_pool, \
             tc.tile_pool(name="bpool", bufs=2, space="SBUF") as bsc_pool, \
             tc.tile_pool(name="obpool", bufs=2, space="SBUF") as ob_pool:

            wq_sb = sb.tile([128, 2048], XDT)
            wk_sb = sb.tile([128, 2048], XDT)
            wv_sb = sb.tile([128, 2048], XDT)
            wo_sb = sb.tile([128, 2048], R32)
            bqT_sb = sb.tile([128, 2], F32)
            bkT_sb = sb.tile([128, 2], F32)
            ones_sb = sb.tile([128, 64], R32)
            qT_sb = sb.tile([128, 4096], R32)
            kT_sb = sb.tile([128, 4096], R32)
            attnT_sb = sb.tile([128, 2048], R32)
            vaug_sb = sb.tile([128, 16 * 260], R32)

            for dc in range(8):
                nc.sync.dma_start(wq_sb[:, dc * 256:(dc + 1) * 256],
                                  aps["wq"][dc * 128:(dc + 1) * 128, :])
            nc.sync.dma_start(bqT_sb[:], aps["bqT"][:])
            # memset cannot target fp32r; stage 1.0s through an F32 tile.
            onesF = sb.tile([128, 64], F32)
            nc.vector.memset(onesF[:], 1.0)
            nc.vector.tensor_copy(ones_sb[:], onesF[:])
            # v_aug row layout per key-chunk j (260 cols): 4 x [v(64) 1]
            vj = vaug_sb[:].rearrange("p (j r) -> p j r", r=260)
            for c in (64, 129, 194, 259):
                nc.vector.tensor_copy(vj[:, :, c:c + 1],
                                      onesF[:, 0:16].unsqueeze(2))

            # ---- q / v / k projections share one 8-bank PSUM ring ----
            with tc.tile_pool(name="projp", bufs=8, space="PSUM") as pp:
                qps = [pp.tile([128, 512], F32, tag="pp", name=f"qp{i}")
                       for i in range(8)]
                for dc in range(8):
                    xt = xp.tile([128, 2048], XDT, tag="xs", name=f"xq{dc}")
                    nc.sync.dma_start(xt[:], xq[dc * 128:(dc + 1) * 128, :])
                    for cc in range(2):
                        for sc in range(4):
                            nc.tensor.matmul(
                                qps[cc * 4 + sc][:],
                                wq_sb[:, dc * 256 + cc * 128:dc * 256 + cc * 128 + 128],
                                xt[:, sc * 512:(sc + 1) * 512],
                                start=(dc == 0), stop=(dc == 7))
                for dc in range(8):
                    nc.sync.dma_start(wv_sb[:, dc * 256:(dc + 1) * 256],
                                      aps["wv"][dc * 128:(dc + 1) * 128, :])
                xvt = []
                for dc in range(8):
                    xt = xvp.tile([128, 2048], XDT, tag="xv", name=f"xv{dc}")
                    nc.sync.dma_start(xt[:], xv[dc * 128:(dc + 1) * 128, :])
                    xvt.append(xt)
                for cc in range(2):
                    for sc in range(4):
                        nc.scalar.add(
                            qT_sb[:, cc * 2048 + sc * 512:cc * 2048 + sc * 512 + 512],
                            qps[cc * 4 + sc][:], bqT_sb[:, cc:cc + 1])

                # v: dc-outer so each resident xv chunk is consumed on arrival
                for half in range(2):
                    vps = [pp.tile([128, 256], F32, tag="pp",
                                   name=f"vp{half}_{i}") for i in range(8)]
                    for dc in range(8):
                        for i in range(8):
                            jj = half * 8 + i
                            nc.tensor.matmul(
                                vps[i][:],
                                xvt[dc][:, jj * 128:(jj + 1) * 128],
                                wv_sb[:, dc * 256:(dc + 1) * 256],
                                start=(dc == 0), stop=(dc == 7))
                    if half == 0:
                        for dc in range(8):
                            nc.sync.dma_start(wk_sb[:, dc * 256:(dc + 1) * 256],
                                              aps["wk"][dc * 128:(dc + 1) * 128, :])
                        nc.sync.dma_start(bkT_sb[:], aps["bkT"][:])
                    for i in range(8):
                        jj = half * 8 + i
                        base = jj * 260
                        dst = vaug_sb[:, base:base + 260] \
                            .rearrange("p (g c) -> p g c", c=65)[:, :, 0:64]
                        nc.vector.tensor_copy(
                            dst, vps[i][:].rearrange("p (g c) -> p g c", c=64))

                kps = [pp.tile([128, 512], F32, tag="pp", name=f"kp{i}")
                       for i in range(8)]
                for dc in range(8):
                    xt = xp.tile([128, 2048], XDT, tag="xs", name=f"xk{dc}")
                    nc.sync.dma_start(xt[:], xk[dc * 128:(dc + 1) * 128, :])
                    for cc in range(2):
                        for sc in range(4):
                            nc.tensor.matmul(
                                kps[cc * 4 + sc][:],
                                wk_sb[:, dc * 256 + cc * 128:dc * 256 + cc * 128 + 128],
                                xt[:, sc * 512:(sc + 1) * 512],
                                start=(dc == 0), stop=(dc == 7))
                for cc in range(2):
                    for sc in range(4):
                        nc.scalar.add(
                            kT_sb[:, cc * 2048 + sc * 512:cc * 2048 + sc * 512 + 512],
                            kps[cc * 4 + sc][:], bkT_sb[:, cc:cc + 1])
                for p in range(2):
                    nc.sync.dma_start(wo_sb[:, p * 1024:(p + 1) * 1024],
                                      aps["wo"][p * 128:(p + 1) * 128, :])

            # ---- attention + normalization + output projection ----
            with tc.tile_pool(name="sp", bufs=2, space="PSUM") as sp, \
                 tc.tile_pool(name="opp", bufs=2, space="PSUM") as opp, \
                 tc.tile_pool(name="vup", bufs=2, space="PSUM") as vup:

                def norm(p, qoff, qlen, poA, poB):
                    z = z_pool.tile([128, 1024], R32, tag="z",
                                    name=f"z{p}_{qoff}")
                    nc.vector.reciprocal(z[64:65, 0:qlen], poA[64:65, 0:qlen])
                    nc.vector.reciprocal(z[64:65, 512:512 + qlen],
                                         poB[64:65, 0:qlen])
                    zzA = vup.tile([128, 512], F32, tag="v",
                                   name=f"zzA{p}_{qoff}")
                    nc.tensor.matmul(zzA[0:64, 0:qlen], ones_sb[64:65, :],
                                     z[64:65, 0:qlen], start=True, stop=True)
                    zzB = vup.tile([128, 512], F32, tag="v",
                                   name=f"zzB{p}_{qoff}")
                    nc.tensor.matmul(zzB[0:64, 0:qlen], ones_sb[64:65, :],
                                     z[64:65, 512:512 + qlen],
                                     start=True, stop=True)
                    # DVE tensor_tensor cannot read two PSUM operands;
                    # stage the 1/Z broadcast through SBUF.
                    zbsA = z_pool.tile([64, 512], F32, tag="zbs",
                                       name=f"zbA{p}_{qoff}")
                    nc.vector.tensor_copy(zbsA[:, 0:qlen], zzA[0:64, 0:qlen])
                    nc.vector.tensor_mul(attnT_sb[0:64, qoff:qoff + qlen],
                                         poA[0:64, 0:qlen], zbsA[:, 0:qlen])
                    zbsB = z_pool.tile([64, 512], F32, tag="zbs",
                                       name=f"zbB{p}_{qoff}")
                    nc.vector.tensor_copy(zbsB[:, 0:qlen], zzB[0:64, 0:qlen])
                    bsc = bsc_pool.tile([64, 512], R32, tag="b",
                                        name=f"bsc{p}_{qoff}")
                    nc.vector.tensor_mul(bsc[:, 0:qlen], poB[0:64, 0:qlen],
                                         zbsB[:, 0:qlen])
                    nc.sync.dma_start(attnT_sb[64:128, qoff:qoff + qlen],
                                      bsc[:, 0:qlen])

                def outproj(p, qoff, qlen):
                    for t in range(qoff // 128, (qoff + qlen) // 128):
                        for n in range(2):
                            up = vup.tile([128, 512], F32, tag="v",
                                          name=f"u{p}{t}{n}")
                            nc.tensor.matmul(
                                up[:],
                                attnT_sb[:, t * 128:(t + 1) * 128],
                                wo_sb[:, p * 1024 + n * 512:p * 1024 + n * 512 + 512],
                                start=True, stop=True)
                            ob = ob_pool.tile([128, 512], F32, tag="ob",
                                              name=f"ob{p}{t}{n}")
                            nc.vector.tensor_copy(ob[:], up[:])
                            nc.sync.dma_start(
                                out_aps[p][t * 128:(t + 1) * 128,
                                           n * 512:(n + 1) * 512],
                                ob[:])

                blocks = [(p, Q * 512, 512) for p in range(2) for Q in range(4)]
                blocks = blocks[:-1] + [(1, 1536, 256), (1, 1792, 256)]
                pend_norm = None
                pend_out = None
                for bi, (p, qoff, qlen) in enumerate(blocks):
                    qb = p * 2048 + qoff
                    pBigs = [None] * 16
                    po = [None, None]

                    def attnv(j, p=p, qlen=qlen, pBigs=pBigs, po=po):
                        va = j * 260 + 2 * p * 65
                        nc.tensor.matmul(po[0][0:65, 0:qlen],
                                         vaug_sb[:, va:va + 65],
                                         pBigs[j][:, 0:qlen],
                                         start=(j == 0), stop=(j == 15))
                        nc.tensor.matmul(po[1][0:65, 0:qlen],
                                         vaug_sb[:, va + 65:va + 130],
                                         pBigs[j][:, qlen:2 * qlen],
                                         start=(j == 0), stop=(j == 15))

                    for j in range(16):
                        kb = p * 2048 + j * 128
                        sBig = sp.tile([128, 1024], F32, tag="s",
                                       name=f"s{bi}_{j}")
                        nc.tensor.matmul(sBig[:, 0:qlen],
                                         kT_sb[0:64, kb:kb + 128],
                                         qT_sb[0:64, qb:qb + qlen],
                                         start=True, stop=True)
                        nc.tensor.matmul(sBig[:, 512:512 + qlen],
                                         kT_sb[64:128, kb:kb + 128],
                                         qT_sb[64:128, qb:qb + qlen],
                                         start=True, stop=True)
                        pBigs[j] = pa_pool.tile([128, 1024], R32, tag="pa",
                                                name=f"pb{bi}_{j}")
                        if qlen == 512:
                            nc.scalar.activation(pBigs[j][:], sBig[:], EXP,
                                                 scale=0.125)
                        else:
                            sv = sBig[:].rearrange("p (g c) -> p g c",
                                                   c=512)[:, :, 0:qlen]
                            pv = pBigs[j][:, 0:2 * qlen] \
                                .rearrange("p (g c) -> p g c", c=qlen)
                            nc.scalar.activation(pv, sv, EXP, scale=0.125)
                        if j == 1:
                            if pend_norm is not None:
                                pend_norm()
                                pend_norm = None
                            po[0] = opp.tile([128, 512], F32, tag="o",
                                             name=f"poA{bi}")
                            po[1] = opp.tile([128, 512], F32, tag="o",
                                             name=f"poB{bi}")
                        if j > 0:
                            attnv(j - 1)
                        if j == 4 and pend_out is not None:
                            pend_out()
                            pend_out = None
                    attnv(15)
                    pend_norm = (lambda p=p, qoff=qoff, qlen=qlen,
                                 A=po[0], B=po[1]: norm(p, qoff, qlen, A, B))
                    pend_out = (lambda p=p, qoff=qoff, qlen=qlen:
                                outproj(p, qoff, qlen))
                pend_norm()
                pend_out()


_NC = None


def _get_nc():
    global _NC
    if _NC is None:
        nc = bacc.Bacc("TRN2", target_bir_lowering=False, debug=False,
                       enable_asserts=False, num_devices=8)
        aps = {}
        for nm, shp in [("xqT", (D, S)), ("xkT", (D, S)), ("xvT", (D, S)),
                        ("wq", (D, 256)), ("wk", (D, 256)), ("wv", (D, 256))]:
            aps[nm] = nc.dram_tensor(nm, shp, XDT, kind="ExternalInput").ap()
        aps["wo"] = nc.dram_tensor("wo", (256, D), R32, kind="ExternalInput").ap()
        for nm, shp in [("bqT", (128, 2)), ("bkT", (128, 2))]:
            aps[nm] = nc.dram_tensor(nm, shp, F32, kind="ExternalInput").ap()
        for nm in ("out_a", "out_b"):
            aps[nm] = nc.dram_tensor(nm, (S, D), F32, kind="ExternalOutput").ap()
        _emit(nc, aps)
        nc.compile()
        _NC = nc
    return _NC


def _run(inputs, trace=False):
    nc = _get_nc()
    f = np.float32
    bf = ml_dtypes.bfloat16
    q = np.asarray(inputs["query"], dtype=f)
    k = np.asarray(inputs["key"], dtype=f)
    v = np.asarray(inputs["value"], dtype=f)
    Wq = np.asarray(inputs["Wq"], dtype=f)
    Wk = np.asarray(inputs["Wk"], dtype=f)
    Wv = np.asarray(inputs["Wv"], dtype=f)
    Wo = np.asarray(inputs["Wo"], dtype=f)
    bq = np.asarray(inputs["bq"], dtype=f)
    bk = np.asarray(inputs["bk"], dtype=f)
    bv = np.asarray(inputs["bv"], dtype=f)
    bo = np.asarray(inputs["bo"], dtype=f)

    xT = {b: (np.ascontiguousarray(q[b].T).astype(bf),
              np.ascontiguousarray(k[b].T).astype(bf),
              np.ascontiguousarray(v[b].T).astype(bf)) for b in range(B)}
    in_maps = []
    for i in range(8):
        b, hg = divmod(i, 4)
        c0 = hg * 256
        in_maps.append({
            "xqT": xT[b][0], "xkT": xT[b][1], "xvT": xT[b][2],
            "wq": np.ascontiguousarray(Wq[:, c0:c0 + 256]).astype(bf),
            "wk": np.ascontiguousarray(Wk[:, c0:c0 + 256]).astype(bf),
            "wv": np.ascontiguousarray(Wv[:, c0:c0 + 256]).astype(bf),
            "bqT": np.ascontiguousarray(bq[c0:c0 + 256].reshape(2, 128).T),
            "bkT": np.ascontiguousarray(bk[c0:c0 + 256].reshape(2, 128).T),
            "wo": np.ascontiguousarray(Wo[c0:c0 + 256, :]),
        })

    res = bass_utils.run_bass_kernel_spmd(nc, in_maps, core_ids=list(range(8)),
                                          trace=trace)
    out = np.zeros((B, S, D), dtype=f)
    for i in range(8):
        out[i // 4] += np.asarray(res.results[i]["out_a"])
        out[i // 4] += np.asarray(res.results[i]["out_b"])
    out += (bv @ Wo + bo)[None, None, :]
    return out, res


def kernel(**inputs):
    out, _ = _run(inputs, trace=False)
    return out
